# revision 1
# baseline (speedup 1.0000x reference)
"""Trainium2 Bass kernel for nn_MetaMultiLinear.

Math (per head h, sample b):
    w[b, k]   = sum_c cond[b, c] * CW[k, c] + cb[k]        k = o*17 + i  (544)
    out[b, o] = sum_i x1[b, i] * w[b, o*17+i]              x1 = [input, 1] (17)

Sharding: head h -> NeuronCore h (8 heads, 8 cores), full B=32768 per core.

Design (final, ~220us HW; baseline v1 was ~454us):
  - The host supplies cond1^T pre-transposed in bf16 (two halves, at
    SBUF partitions 0-32 and 64-96), so no on-device transpose is
    needed; weights cwk/cwo and the identity are bf16 too (bf16 LDW is
    ~2.7x cheaper than fp32r and the extra rounding keeps rel err at
    ~2.6e-3, well under the 2e-2 gate).
  - Per tile of 128 samples (processed in pairs: tile A's stationary at
    partitions 0-32 / tile_position (0,0), tile B at 64-96 / (64,0), so
    both quadrants hold weights concurrently):  W-MM
    w[b, o*16+i] = cond1 @ cwk^T (K=33, N=512, one PSUM bank per tile)
    and po-MM po[b, o] = cond1 @ cwo (N=32, start=True stop=False,
    opens that tile's po accumulation bank, carries bias + i=16 terms).
    po tiles of a pair must be in SEPARATE banks (sharing one bank
    raises a device error).
  - DVE (the floor, ~158us): one tensor_mul per pair over both W PSUM
    banks: tmp[b, t, i, o] = w (*) broadcast(x), 1024 elem/partition,
    bf16 out.  PSUM input forces 1x mode; nothing else can read PSUM
    fast (ACT ~1ns/elem, GPSIMD has no PSUM port), so this is a hard
    ~1.2us/pair.
  - Per tile one reduce matmul (bf16, N=512, contiguous rhs): identity
    stationary streams tmp i-outer/o-inner; the PSUM out AP broadcasts
    over i so 16 passes accumulate onto po[b, o] via has_written.
    Reduces run one pair behind so the PE always has W work queued.
    (Reduce on DVE instead stalls the mul FIFO -- net loss.)
  - ScalarE copies po -> SBUF; one output DMA per group of 16 pairs
    (first/last groups split finer to shrink ramp/tail); the host
    un-permutes tiles.  Consts/x DMAs issue on the ScalarE DGE queue to
    parallelize the startup ramp.
  - PE is the wall (~200us active at the observed 1.2 GHz effective
    clock; ~80us of that is per-matmul LDWEIGHTS, unavoidable since
    walrus' ldw-opt pass crashes and fp32r/bf16 matmuls self-load).
"""

import sys

import numpy as np

if "/opt/trn_rl_repo" not in sys.path:
    sys.path.insert(0, "/opt/trn_rl_repo")

N_HEADS, IN_F, COND_IN, OUT_F = 8, 16, 32, 32
B = 32768
INP1 = IN_F + 1  # 17
KW = OUT_F * IN_F  # 512 (i<16 part)
C1 = COND_IN + 1  # 33
P = 128
GROUPS = 8
PAIRS_PER_GROUP = B // (2 * P) // GROUPS  # 16
GCOLS = B // (2 * GROUPS)  # 2048 cond1T columns per group half

_cached_nc = None

USE_F32R = True
# "overlap": PE grouped reduce via overlapping PSUM out-AP (1 matmul/pair)
# "mm16":    PE grouped reduce via 16 accumulated strided matmuls (sim-safe)
REDUCE_MODE = "overlap"
# every DVE_RED_MOD-th pair reduces on the DVE instead of the PE, to
# balance the two engines (0 disables)
DVE_RED_MOD = 0


def _build_nc():
    import concourse.mybir as mybir
    import concourse.tile as tile
    from concourse import bacc
    from contextlib import ExitStack

    f32 = mybir.dt.float32
    bf16 = mybir.dt.bfloat16
    fr = mybir.dt.float32r if USE_F32R else f32
    nc = bacc.Bacc()

    # ct[r, g*2048+s]: r<33 -> cond1T[r, g*4096+s]; r>=33 -> cond1T[r-33, g*4096+2048+s]
    ct_t = nc.dram_tensor("ct", [2 * C1, GROUPS * GCOLS], bf16, kind="ExternalInput")
    # x[p, ((g j) t) i] = input[g*4096 + t*2048 + j*128 + p, i]
    x_t = nc.dram_tensor("x", [P, B // P * IN_F], f32, kind="ExternalInput")
    # cwk[c, o*16+i] = CW[o*17+i, c] (i<16); row 32 = cond_bias slice; rows 64-96 repeat
    cwk_t = nc.dram_tensor("cwk", [P, KW], bf16, kind="ExternalInput")
    # cwo[c, o] = CW[o*17+16, c]; row 32 = cond_bias[o*17+16]; rows 64-96 repeat
    cwo_t = nc.dram_tensor("cwo", [P, OUT_F], bf16, kind="ExternalInput")
    ident_t = nc.dram_tensor("ident", [P, P], bf16, kind="ExternalInput")
    # out[p, ((g j) t) o] = out[g*4096 + t*2048 + j*128 + p, o]
    out_t = nc.dram_tensor("out", [P, B // P * OUT_F], f32, kind="ExternalOutput")

    with tile.TileContext(nc) as tc, ExitStack() as ctx:
        consts = ctx.enter_context(tc.tile_pool(name="consts", bufs=1))
        pct = ctx.enter_context(tc.tile_pool(name="pct", bufs=3))
        px = ctx.enter_context(tc.tile_pool(name="px", bufs=3))
        pouts = ctx.enter_context(tc.tile_pool(name="pouts", bufs=3))
        ptmp = ctx.enter_context(tc.tile_pool(name="ptmp", bufs=4))
        pres = ctx.enter_context(tc.tile_pool(name="pres", bufs=2))
        ppw = ctx.enter_context(tc.tile_pool(name="ppw", bufs=2, space="PSUM"))
        ppo = ctx.enter_context(tc.tile_pool(name="ppo", bufs=2, space="PSUM"))

        cwk = consts.tile([P, KW], bf16)
        nc.scalar.dma_start(out=cwk, in_=cwk_t[:])
        cwo = consts.tile([P, OUT_F], bf16)
        nc.scalar.dma_start(out=cwo, in_=cwo_t[:])
        idn = consts.tile([P, P], bf16)
        nc.scalar.dma_start(out=idn, in_=ident_t[:])

        pending = []  # (po, tmp, outs_g, j, g)

        def emit_out_dma(outs_g, g, j0, j1):
            nc.sync.dma_start(
                out=out_t[
                    :,
                    (g * PAIRS_PER_GROUP + j0) * 2 * OUT_F : (g * PAIRS_PER_GROUP + j1)
                    * 2
                    * OUT_F,
                ].rearrange("p (j t o) -> p j t o", j=j1 - j0, t=2),
                in_=outs_g[:, 0 : j1 - j0],
            )

        def emit_reduce(item):
            po, tmp, outs_g, j, g, j0, j1, nj, dve_red = item
            if dve_red:
                # balance engines: reduce this pair on the DVE instead.
                # tmp was written [p, t, o, i] (i packed innermost).
                pred = pres.tile([P, 2, OUT_F], f32)
                nc.vector.reduce_sum(pred[:], tmp[:], axis=mybir.AxisListType.X)
                nc.scalar.copy(out=outs_g[:, j], in_=po[:, :, 0:OUT_F])
                nc.vector.tensor_add(outs_g[:, j], pred[:], outs_g[:, j])
                if j == nj - 1:
                    emit_out_dma(outs_g, g, j0, j1)
                return
            for t in (0, 1):
                # i-outer / o-inner, fully contiguous rhs; 16 passes of 32
                # o-columns accumulate onto po[t*32+o] via has_written.
                rhs = tmp[:, t]
                if REDUCE_MODE == "overlap":
                    ov = (
                        po[:, t, 0:OUT_F]
                        .unsqueeze(1)
                        .broadcast_to([P, IN_F, OUT_F])
                    )
                    nc.tensor.matmul(
                        ov,
                        idn[:],
                        rhs,
                        start=False,
                        stop=True,
                        skip_group_check=True,
                    )
                else:
                    for i in range(IN_F):
                        nc.tensor.matmul(
                            po[:, t, 0:OUT_F],
                            idn[:],
                            rhs[:, i, :],
                            start=False,
                            stop=(i == IN_F - 1),
                            skip_group_check=True,
                        )
            nc.scalar.copy(out=outs_g[:, j], in_=po[:, :, 0:OUT_F])
            if j == nj - 1:
                emit_out_dma(outs_g, g, j0, j1)

        segments = []
        for g in range(GROUPS):
            if g == 0:
                segments += [(g, 0, 2), (g, 2, 4), (g, 4, 8), (g, 8, PAIRS_PER_GROUP)]
            elif g == GROUPS - 1:
                segments += [(g, 0, 8), (g, 8, 12), (g, 12, 14), (g, 14, PAIRS_PER_GROUP)]
            else:
                segments.append((g, 0, PAIRS_PER_GROUP))
        for g, j0, j1 in segments:
            nj = j1 - j0
            c0 = g * GCOLS + j0 * P
            c1 = g * GCOLS + j1 * P
            ct_g = pct.tile([P, GCOLS], bf16, tag="ct_g")
            nc.sync.dma_start(out=ct_g[0:C1, 0 : nj * P], in_=ct_t[0:C1, c0:c1])
            nc.sync.dma_start(
                out=ct_g[64 : 64 + C1, 0 : nj * P],
                in_=ct_t[C1 : 2 * C1, c0:c1],
            )
            x_g = px.tile([P, PAIRS_PER_GROUP, 2, IN_F], f32, tag="x_g")
            # very first segment: keep x off the scalar queue (behind consts)
            x_eng = nc.sync if (g == 0 and j0 == 0) else nc.scalar
            x_eng.dma_start(
                out=x_g[:, 0:nj],
                in_=x_t[
                    :, (g * PAIRS_PER_GROUP + j0) * 2 * IN_F : (g * PAIRS_PER_GROUP + j1) * 2 * IN_F
                ].rearrange("p (j t i) -> p j t i", j=nj, t=2),
            )
            outs_g = pouts.tile([P, PAIRS_PER_GROUP, 2, OUT_F], f32, tag="outs_g")

            for j in range(nj):
                gp = g * PAIRS_PER_GROUP + j0 + j
                dve_red = DVE_RED_MOD > 0 and gp % DVE_RED_MOD == DVE_RED_MOD // 2
                wpair = ppw.tile([P, 2, KW], f32)
                po = ppo.tile([P, 2, 512], f32)
                for t, g0 in enumerate((0, 64)):
                    cts = ct_g[g0 : g0 + C1, j * P : (j + 1) * P]
                    nc.tensor.matmul(
                        wpair[:, t, :],
                        cts,
                        cwk[g0 : g0 + C1, :],
                        start=True,
                        stop=True,
                        tile_position=(g0, 0),
                    )
                    # opens tile t's po accumulation bank for the PE reduce
                    nc.tensor.matmul(
                        po[:, t, 0:OUT_F],
                        cts,
                        cwo[g0 : g0 + C1, :],
                        start=True,
                        stop=dve_red,
                        skip_group_check=True,
                        tile_position=(g0, 0),
                    )
                if dve_red:
                    # [p, t, o, i]: packed i innermost for the DVE reduce
                    tmp = ptmp.tile([P, 2, OUT_F, IN_F], bf16)
                    wview = wpair[:].rearrange("p t (o i) -> p t o i", i=IN_F)
                    xv = x_g[:, j].unsqueeze(2).broadcast_to([P, 2, OUT_F, IN_F])
                else:
                    # [p, t, i, o]: contiguous i-pass rhs for the PE reduce
                    tmp = ptmp.tile([P, 2, IN_F, OUT_F], bf16)
                    wview = wpair[:].rearrange("p t (o i) -> p t i o", i=IN_F)
                    xv = x_g[:, j].unsqueeze(3).broadcast_to([P, 2, IN_F, OUT_F])
                nc.vector.tensor_mul(tmp[:], wview, xv)
                pending.append((po, tmp, outs_g, j, g, j0, j1, nj, dve_red))
                if len(pending) > 1:
                    emit_reduce(pending.pop(0))
        while pending:
            emit_reduce(pending.pop(0))

    nc.compile()
    return nc


def _get_nc():
    global _cached_nc
    if _cached_nc is None:
        _cached_nc = _build_nc()
    return _cached_nc


def _make_in_maps(input, cond, cond_weight, cond_bias):
    import ml_dtypes

    bf = ml_dtypes.bfloat16
    ident = np.eye(P, dtype=bf)
    in_maps = []
    n_heads, b_total = input.shape[0], input.shape[1]
    for h in range(n_heads):
        c1t = np.empty((C1, b_total), np.float32)
        c1t[:COND_IN] = cond[h].T
        c1t[COND_IN] = 1.0
        # [33, g, t, s] -> [t, 33, g, s] -> [66, g*s]
        ct = (
            c1t.reshape(C1, GROUPS, 2, GCOLS)
            .transpose(2, 0, 1, 3)
            .reshape(2 * C1, GROUPS * GCOLS)
        )
        ct = np.ascontiguousarray(ct)
        # x[p, (g j t i)] = input[g*4096 + t*2048 + j*128 + p, i]
        x = (
            input[h]
            .reshape(GROUPS, 2, PAIRS_PER_GROUP, P, IN_F)
            .transpose(3, 0, 2, 1, 4)
            .reshape(P, b_total // P * IN_F)
        )
        x = np.ascontiguousarray(x)
        cw3 = cond_weight[h].reshape(OUT_F, INP1, COND_IN)  # (o, i, c)
        cb2 = cond_bias[h].reshape(OUT_F, INP1)  # (o, i)
        cwk = np.zeros((P, KW), np.float32)
        cwk1 = cw3[:, :IN_F, :].transpose(2, 0, 1).reshape(COND_IN, KW)
        cwk[0:COND_IN] = cwk1
        cwk[COND_IN] = cb2[:, :IN_F].reshape(KW)
        cwk[64 : 64 + COND_IN] = cwk1
        cwk[64 + COND_IN] = cb2[:, :IN_F].reshape(KW)
        cwo = np.zeros((P, OUT_F), np.float32)
        cwo[0:COND_IN] = cw3[:, IN_F, :].T  # [c, o]
        cwo[COND_IN] = cb2[:, IN_F]
        cwo[64 : 64 + COND_IN] = cw3[:, IN_F, :].T
        cwo[64 + COND_IN] = cb2[:, IN_F]
        in_maps.append(
            {
                "ct": ct.astype(bf),
                "x": x,
                "cwk": cwk.astype(bf),
                "cwo": cwo.astype(bf),
                "ident": ident,
            }
        )
    return in_maps


def _unpack_out(res):
    # out[p, (g j t o)] -> [g, t, j, p, o] -> [B, o]
    outs = []
    for r in res.results:
        o = (
            r["out"]
            .reshape(P, GROUPS, PAIRS_PER_GROUP, 2, OUT_F)
            .transpose(1, 3, 2, 0, 4)
            .reshape(B, OUT_F)
        )
        outs.append(o)
    return np.stack(outs, axis=0)


def _run(in_maps, **kwargs):
    from concourse import bass_utils

    nc = _get_nc()
    return bass_utils.run_bass_kernel_spmd(
        nc, in_maps, core_ids=list(range(N_HEADS)), **kwargs
    )


def kernel(input, cond, cond_weight, cond_bias):
    input = np.asarray(input, np.float32)
    cond = np.asarray(cond, np.float32)
    cond_weight = np.asarray(cond_weight, np.float32)
    cond_bias = np.asarray(cond_bias, np.float32)
    in_maps = _make_in_maps(input, cond, cond_weight, cond_bias)
    res = _run(in_maps)
    return _unpack_out(res)



# revision 7
# speedup vs baseline: 1.1963x; 1.1963x over previous
"""Trainium2 Bass kernel for nn_MetaMultiLinear.

Math (per head h, sample b):
    w[b, k]   = sum_c cond[b, c] * CW[k, c] + cb[k]        k = o*17 + i  (544)
    out[b, o] = sum_i x1[b, i] * w[b, o*17+i]              x1 = [input, 1] (17)

Sharding: head h -> NeuronCore h (8 heads, 8 cores), full B=32768 per core.

Design (final, ~220us HW; baseline v1 was ~454us):
  - The host supplies cond1^T pre-transposed in bf16 (two halves, at
    SBUF partitions 0-32 and 64-96), so no on-device transpose is
    needed; weights cwk/cwo and the identity are bf16 too (bf16 LDW is
    ~2.7x cheaper than fp32r and the extra rounding keeps rel err at
    ~2.6e-3, well under the 2e-2 gate).
  - Per tile of 128 samples (processed in pairs: tile A's stationary at
    partitions 0-32 / tile_position (0,0), tile B at 64-96 / (64,0), so
    both quadrants hold weights concurrently):  W-MM
    w[b, o*16+i] = cond1 @ cwk^T (K=33, N=512, one PSUM bank per tile)
    and po-MM po[b, o] = cond1 @ cwo (N=32, start=True stop=False,
    opens that tile's po accumulation bank, carries bias + i=16 terms).
    po tiles of a pair must be in SEPARATE banks (sharing one bank
    raises a device error).
  - DVE (the floor, ~158us): one tensor_mul per pair over both W PSUM
    banks: tmp[b, t, i, o] = w (*) broadcast(x), 1024 elem/partition,
    bf16 out.  PSUM input forces 1x mode; nothing else can read PSUM
    fast (ACT ~1ns/elem, GPSIMD has no PSUM port), so this is a hard
    ~1.2us/pair.
  - Per tile one reduce matmul (bf16, N=512, contiguous rhs): identity
    stationary streams tmp i-outer/o-inner; the PSUM out AP broadcasts
    over i so 16 passes accumulate onto po[b, o] via has_written.
    Reduces run one pair behind so the PE always has W work queued.
    (Reduce on DVE instead stalls the mul FIFO -- net loss.)
  - ScalarE copies po -> SBUF; one output DMA per group of 16 pairs
    (first/last groups split finer to shrink ramp/tail); the host
    un-permutes tiles.  Consts/x DMAs issue on the ScalarE DGE queue to
    parallelize the startup ramp.
  - PE is the wall (~200us active at the observed 1.2 GHz effective
    clock; ~80us of that is per-matmul LDWEIGHTS, unavoidable since
    walrus' ldw-opt pass crashes and fp32r/bf16 matmuls self-load).
"""

import sys

import numpy as np

if "/opt/trn_rl_repo" not in sys.path:
    sys.path.insert(0, "/opt/trn_rl_repo")

N_HEADS, IN_F, COND_IN, OUT_F = 8, 16, 32, 32
B = 32768
INP1 = IN_F + 1  # 17
KW = OUT_F * IN_F  # 512 (i<16 part)
C1 = COND_IN + 1  # 33
P = 128
GROUPS = 8
PAIRS_PER_GROUP = B // (2 * P) // GROUPS  # 16
GCOLS = B // (2 * GROUPS)  # 2048 cond1T columns per group half

_cached_nc = None

USE_F32R = True
# "overlap": PE grouped reduce via overlapping PSUM out-AP (1 matmul/pair)
# "mm16":    PE grouped reduce via 16 accumulated strided matmuls (sim-safe)
REDUCE_MODE = "overlap"
# every DVE_RED_MOD-th pair reduces on the DVE instead of the PE, to
# balance the two engines (0 disables)
DVE_RED_MOD = 0


def _build_nc():
    import concourse.mybir as mybir
    import concourse.tile as tile
    from concourse import bacc
    from contextlib import ExitStack

    f32 = mybir.dt.float32
    bf16 = mybir.dt.bfloat16
    fr = mybir.dt.float32r if USE_F32R else f32
    nc = bacc.Bacc()

    # ct[r, g*2048+s]: r<33 -> cond1T[r, g*4096+s]; r>=33 -> cond1T[r-33, g*4096+2048+s]
    ct_t = nc.dram_tensor("ct", [2 * C1, GROUPS * GCOLS], bf16, kind="ExternalInput")
    # x[p, ((g j) t) i] = input[g*4096 + t*2048 + j*128 + p, i]
    x_t = nc.dram_tensor("x", [P, B // P * IN_F], f32, kind="ExternalInput")
    # cwk[c, o*16+i] = CW[o*17+i, c] (i<16); row 32 = cond_bias slice; rows 64-96 repeat
    cwk_t = nc.dram_tensor("cwk", [P, KW], bf16, kind="ExternalInput")
    # cwo[c, o] = CW[o*17+16, c]; row 32 = cond_bias[o*17+16]; rows 64-96 repeat
    cwo_t = nc.dram_tensor("cwo", [P, OUT_F], bf16, kind="ExternalInput")
    ident_t = nc.dram_tensor("ident", [P, P], bf16, kind="ExternalInput")
    # out[p, ((g j) t) o] = out[g*4096 + t*2048 + j*128 + p, o]
    out_t = nc.dram_tensor("out", [P, B // P * OUT_F], f32, kind="ExternalOutput")

    with tile.TileContext(nc) as tc, ExitStack() as ctx:
        consts = ctx.enter_context(tc.tile_pool(name="consts", bufs=1))
        pct = ctx.enter_context(tc.tile_pool(name="pct", bufs=3))
        px = ctx.enter_context(tc.tile_pool(name="px", bufs=3))
        pouts = ctx.enter_context(tc.tile_pool(name="pouts", bufs=3))
        ptmp = ctx.enter_context(tc.tile_pool(name="ptmp", bufs=4))
        pres = ctx.enter_context(tc.tile_pool(name="pres", bufs=2))
        ppw = ctx.enter_context(tc.tile_pool(name="ppw", bufs=2, space="PSUM"))
        ppo = ctx.enter_context(tc.tile_pool(name="ppo", bufs=2, space="PSUM"))

        cwk = consts.tile([P, KW], bf16)
        nc.scalar.dma_start(out=cwk, in_=cwk_t[:])
        cwo = consts.tile([P, OUT_F], bf16)
        nc.scalar.dma_start(out=cwo, in_=cwo_t[:])
        idn = consts.tile([P, P], bf16)
        nc.scalar.dma_start(out=idn, in_=ident_t[:])

        pending = []  # (po, tmp, cts01, outs_g, j, g, ...)

        def emit_out_dma(outs_g, g, j0, j1):
            nc.sync.dma_start(
                out=out_t[
                    :,
                    (g * PAIRS_PER_GROUP + j0) * 2 * OUT_F : (g * PAIRS_PER_GROUP + j1)
                    * 2
                    * OUT_F,
                ].rearrange("p (j t o) -> p j t o", j=j1 - j0, t=2),
                in_=outs_g[:, 0 : j1 - j0],
            )

        def emit_reduce(item):
            po, tmp, cts01, outs_g, j, g, j0, j1, nj, dve_red = item
            if dve_red:
                # balance engines: reduce this pair on the DVE instead.
                # tmp was written [p, t, o, i] (i packed innermost).
                pred = pres.tile([P, 2, OUT_F], f32)
                nc.vector.reduce_sum(pred[:], tmp[:], axis=mybir.AxisListType.X)
                nc.scalar.copy(out=outs_g[:, j], in_=po[:, :, 0:OUT_F])
                nc.vector.tensor_add(outs_g[:, j], pred[:], outs_g[:, j])
                if j == nj - 1:
                    emit_out_dma(outs_g, g, j0, j1)
                return
            # Two matmuls over BOTH tiles (i halves): rhs rows are ordered
            # (i, t, o) -- contiguous in tmp -- so the same PSUM address is
            # revisited every 64 rows instead of 32, clearing the
            # accumulate RMW-hazard stall.  (ISA: out num_elements <= 512.)
            for h in (0, 1):
                ov = (
                    po[:, :, 0:OUT_F]
                    .unsqueeze(1)
                    .broadcast_to([P, IN_F // 2, 2, OUT_F])
                )
                nc.tensor.matmul(
                    ov,
                    idn[:],
                    tmp[:, h * (IN_F // 2) : (h + 1) * (IN_F // 2)].rearrange(
                        "p i t o -> p (i t o)"
                    ),
                    start=False,
                    stop=(h == 1),
                    skip_group_check=True,
                )
            nc.scalar.copy(out=outs_g[:, j], in_=po[:, :, 0:OUT_F])
            if j == nj - 1:
                emit_out_dma(outs_g, g, j0, j1)

        segments = []
        for g in range(GROUPS):
            if g == 0:
                segments += [(g, 0, 2), (g, 2, 4), (g, 4, 8), (g, 8, PAIRS_PER_GROUP)]
            elif g == GROUPS - 1:
                segments += [(g, 0, 8), (g, 8, 12), (g, 12, 14), (g, 14, PAIRS_PER_GROUP)]
            else:
                segments.append((g, 0, PAIRS_PER_GROUP))
        for g, j0, j1 in segments:
            nj = j1 - j0
            c0 = g * GCOLS + j0 * P
            c1 = g * GCOLS + j1 * P
            ct_g = pct.tile([P, GCOLS], bf16, tag="ct_g")
            nc.sync.dma_start(out=ct_g[0:C1, 0 : nj * P], in_=ct_t[0:C1, c0:c1])
            nc.sync.dma_start(
                out=ct_g[64 : 64 + C1, 0 : nj * P],
                in_=ct_t[C1 : 2 * C1, c0:c1],
            )
            x_g = px.tile([P, PAIRS_PER_GROUP, 2, IN_F], f32, tag="x_g")
            # very first segment: keep x off the scalar queue (behind consts)
            x_eng = nc.sync if (g == 0 and j0 == 0) else nc.scalar
            x_eng.dma_start(
                out=x_g[:, 0:nj],
                in_=x_t[
                    :, (g * PAIRS_PER_GROUP + j0) * 2 * IN_F : (g * PAIRS_PER_GROUP + j1) * 2 * IN_F
                ].rearrange("p (j t i) -> p j t i", j=nj, t=2),
            )
            outs_g = pouts.tile([P, PAIRS_PER_GROUP, 2, OUT_F], f32, tag="outs_g")

            for j in range(nj):
                gp = g * PAIRS_PER_GROUP + j0 + j
                dve_red = DVE_RED_MOD > 0 and gp % DVE_RED_MOD == DVE_RED_MOD // 2
                wpair = ppw.tile([P, 2, KW], f32)
                po = ppo.tile([P, 2, 512], f32)
                cts01 = []
                for t, g0 in enumerate((0, 64)):
                    cts = ct_g[g0 : g0 + C1, j * P : (j + 1) * P]
                    cts01.append(cts)
                    nc.tensor.matmul(
                        wpair[:, t, :],
                        cts,
                        cwk[g0 : g0 + C1, :],
                        start=True,
                        stop=True,
                        tile_position=(g0, 0),
                    )
                    # opens tile t's po accumulation bank for the PE reduce
                    nc.tensor.matmul(
                        po[:, t, 0:OUT_F],
                        cts,
                        cwo[g0 : g0 + C1, :],
                        start=True,
                        stop=dve_red,
                        skip_group_check=True,
                        tile_position=(g0, 0),
                    )
                if dve_red:
                    # [p, t, o, i]: packed i innermost for the DVE reduce
                    tmp = ptmp.tile([P, 2, OUT_F, IN_F], bf16)
                    wview = wpair[:].rearrange("p t (o i) -> p t o i", i=IN_F)
                    xv = x_g[:, j].unsqueeze(2).broadcast_to([P, 2, OUT_F, IN_F])
                else:
                    # [p, i, t, o]: contiguous (i t o) rhs for the PE reduce
                    tmp = ptmp.tile([P, IN_F, 2, OUT_F], bf16)
                    wview = wpair[:].rearrange("p t (o i) -> p i t o", i=IN_F)
                    xv = (
                        x_g[:, j]
                        .rearrange("p t i -> p i t")
                        .unsqueeze(3)
                        .broadcast_to([P, IN_F, 2, OUT_F])
                    )
                nc.vector.tensor_mul(tmp[:], wview, xv)
                pending.append((po, tmp, cts01, outs_g, j, g, j0, j1, nj, dve_red))
                if len(pending) > 1:
                    emit_reduce(pending.pop(0))
        while pending:
            emit_reduce(pending.pop(0))

    nc.compile()
    return nc


def _get_nc():
    global _cached_nc
    if _cached_nc is None:
        _cached_nc = _build_nc()
    return _cached_nc


def _make_in_maps(input, cond, cond_weight, cond_bias):
    import ml_dtypes

    bf = ml_dtypes.bfloat16
    ident = np.eye(P, dtype=bf)
    in_maps = []
    n_heads, b_total = input.shape[0], input.shape[1]
    for h in range(n_heads):
        c1t = np.empty((C1, b_total), np.float32)
        c1t[:COND_IN] = cond[h].T
        c1t[COND_IN] = 1.0
        # [33, g, t, s] -> [t, 33, g, s] -> [66, g*s]
        ct = (
            c1t.reshape(C1, GROUPS, 2, GCOLS)
            .transpose(2, 0, 1, 3)
            .reshape(2 * C1, GROUPS * GCOLS)
        )
        ct = np.ascontiguousarray(ct)
        # x[p, (g j t i)] = input[g*4096 + t*2048 + j*128 + p, i]
        x = (
            input[h]
            .reshape(GROUPS, 2, PAIRS_PER_GROUP, P, IN_F)
            .transpose(3, 0, 2, 1, 4)
            .reshape(P, b_total // P * IN_F)
        )
        x = np.ascontiguousarray(x)
        cw3 = cond_weight[h].reshape(OUT_F, INP1, COND_IN)  # (o, i, c)
        cb2 = cond_bias[h].reshape(OUT_F, INP1)  # (o, i)
        cwk = np.zeros((P, KW), np.float32)
        cwk1 = cw3[:, :IN_F, :].transpose(2, 0, 1).reshape(COND_IN, KW)
        cwk[0:COND_IN] = cwk1
        cwk[COND_IN] = cb2[:, :IN_F].reshape(KW)
        cwk[64 : 64 + COND_IN] = cwk1
        cwk[64 + COND_IN] = cb2[:, :IN_F].reshape(KW)
        cwo = np.zeros((P, OUT_F), np.float32)
        cwo[0:COND_IN] = cw3[:, IN_F, :].T  # [c, o]
        cwo[COND_IN] = cb2[:, IN_F]
        cwo[64 : 64 + COND_IN] = cw3[:, IN_F, :].T
        cwo[64 + COND_IN] = cb2[:, IN_F]
        in_maps.append(
            {
                "ct": ct.astype(bf),
                "x": x,
                "cwk": cwk.astype(bf),
                "cwo": cwo.astype(bf),
                "ident": ident,
            }
        )
    return in_maps


def _unpack_out(res):
    # out[p, (g j t o)] -> [g, t, j, p, o] -> [B, o]
    outs = []
    for r in res.results:
        o = (
            r["out"]
            .reshape(P, GROUPS, PAIRS_PER_GROUP, 2, OUT_F)
            .transpose(1, 3, 2, 0, 4)
            .reshape(B, OUT_F)
        )
        outs.append(o)
    return np.stack(outs, axis=0)


def _run(in_maps, **kwargs):
    from concourse import bass_utils

    nc = _get_nc()
    return bass_utils.run_bass_kernel_spmd(
        nc, in_maps, core_ids=list(range(N_HEADS)), **kwargs
    )


def kernel(input, cond, cond_weight, cond_bias):
    input = np.asarray(input, np.float32)
    cond = np.asarray(cond, np.float32)
    cond_weight = np.asarray(cond_weight, np.float32)
    cond_bias = np.asarray(cond_bias, np.float32)
    in_maps = _make_in_maps(input, cond, cond_weight, cond_bias)
    res = _run(in_maps)
    return _unpack_out(res)



# revision 13
# speedup vs baseline: 1.2921x; 1.0801x over previous
"""Trainium2 Bass kernel for nn_MetaMultiLinear.

Math (per head h, sample b):
    w[b, k]   = sum_c cond[b, c] * CW[k, c] + cb[k]        k = o*17 + i  (544)
    out[b, o] = sum_i x1[b, i] * w[b, o*17+i]              x1 = [input, 1] (17)

Sharding: head h -> NeuronCore h (8 heads, 8 cores), full B=32768 per core.

Design (final, ~220us HW; baseline v1 was ~454us):
  - The host supplies cond1^T pre-transposed in bf16 (two halves, at
    SBUF partitions 0-32 and 64-96), so no on-device transpose is
    needed; weights cwk/cwo and the identity are bf16 too (bf16 LDW is
    ~2.7x cheaper than fp32r and the extra rounding keeps rel err at
    ~2.6e-3, well under the 2e-2 gate).
  - Per tile of 128 samples (processed in pairs: tile A's stationary at
    partitions 0-32 / tile_position (0,0), tile B at 64-96 / (64,0), so
    both quadrants hold weights concurrently):  W-MM
    w[b, o*16+i] = cond1 @ cwk^T (K=33, N=512, one PSUM bank per tile)
    and po-MM po[b, o] = cond1 @ cwo (N=32, start=True stop=False,
    opens that tile's po accumulation bank, carries bias + i=16 terms).
    po tiles of a pair must be in SEPARATE banks (sharing one bank
    raises a device error).
  - DVE (the floor, ~158us): one tensor_mul per pair over both W PSUM
    banks: tmp[b, t, i, o] = w (*) broadcast(x), 1024 elem/partition,
    bf16 out.  PSUM input forces 1x mode; nothing else can read PSUM
    fast (ACT ~1ns/elem, GPSIMD has no PSUM port), so this is a hard
    ~1.2us/pair.
  - Per tile one reduce matmul (bf16, N=512, contiguous rhs): identity
    stationary streams tmp i-outer/o-inner; the PSUM out AP broadcasts
    over i so 16 passes accumulate onto po[b, o] via has_written.
    Reduces run one pair behind so the PE always has W work queued.
    (Reduce on DVE instead stalls the mul FIFO -- net loss.)
  - ScalarE copies po -> SBUF; one output DMA per group of 16 pairs
    (first/last groups split finer to shrink ramp/tail); the host
    un-permutes tiles.  Consts/x DMAs issue on the ScalarE DGE queue to
    parallelize the startup ramp.
  - PE is the wall (~200us active at the observed 1.2 GHz effective
    clock; ~80us of that is per-matmul LDWEIGHTS, unavoidable since
    walrus' ldw-opt pass crashes and fp32r/bf16 matmuls self-load).
"""

import sys

import numpy as np

if "/opt/trn_rl_repo" not in sys.path:
    sys.path.insert(0, "/opt/trn_rl_repo")

N_HEADS, IN_F, COND_IN, OUT_F = 8, 16, 32, 32
B = 32768
INP1 = IN_F + 1  # 17
KW = OUT_F * IN_F  # 512 (i<16 part)
C1 = COND_IN + 1  # 33
P = 128
GROUPS = 8
PAIRS_PER_GROUP = B // (2 * P) // GROUPS  # 16
GCOLS = B // (2 * GROUPS)  # 2048 cond1T columns per group half

_cached_nc = None

USE_F32R = True
# "overlap": PE grouped reduce via overlapping PSUM out-AP (1 matmul/pair)
# "mm16":    PE grouped reduce via 16 accumulated strided matmuls (sim-safe)
REDUCE_MODE = "overlap"
# every DVE_RED_MOD-th pair reduces on the DVE instead of the PE, to
# balance the two engines (0 disables)
DVE_RED_MOD = 0
# ACT engine copies wpair PSUM->SBUF bf16 so the DVE multiply runs 2x
ACT_COPY = True


def _build_nc():
    import concourse.mybir as mybir
    import concourse.tile as tile
    from concourse import bacc
    from contextlib import ExitStack

    f32 = mybir.dt.float32
    bf16 = mybir.dt.bfloat16
    fr = mybir.dt.float32r if USE_F32R else f32
    nc = bacc.Bacc()

    # ct[r, g*2048+s]: r<33 -> cond1T[r, g*4096+s]; r>=33 -> cond1T[r-33, g*4096+2048+s]
    ct_t = nc.dram_tensor("ct", [2 * C1, GROUPS * GCOLS], bf16, kind="ExternalInput")
    # x[p, ((g j) t) i] = input[g*4096 + t*2048 + j*128 + p, i]
    x_t = nc.dram_tensor("x", [P, B // P * IN_F], bf16, kind="ExternalInput")
    # cwk[c, o*16+i] = CW[o*17+i, c] (i<16); row 32 = cond_bias slice; rows 64-96 repeat
    cwk_t = nc.dram_tensor("cwk", [P, KW], bf16, kind="ExternalInput")
    # cwo[c, o*2] = CW[o*17+16, c]; row 32 = cond_bias[o*17+16]; rows 64-96
    # repeat; odd cols are ZERO (they init the il=1 accumulator slots)
    cwo_t = nc.dram_tensor("cwo", [P, 2 * OUT_F], bf16, kind="ExternalInput")
    # compact cwo (plain [c, o]) for the DVE-reduce path's po
    cwob_t = nc.dram_tensor("cwob", [P, OUT_F], bf16, kind="ExternalInput")
    ident_t = nc.dram_tensor("ident", [P, P], bf16, kind="ExternalInput")
    # out[p, ((g j) t) o] = out[g*4096 + t*2048 + j*128 + p, o]
    out_t = nc.dram_tensor("out", [P, B // P * OUT_F], f32, kind="ExternalOutput")

    with tile.TileContext(nc) as tc, ExitStack() as ctx:
        consts = ctx.enter_context(tc.tile_pool(name="consts", bufs=1))
        pct = ctx.enter_context(tc.tile_pool(name="pct", bufs=3))
        px = ctx.enter_context(tc.tile_pool(name="px", bufs=3))
        pouts = ctx.enter_context(tc.tile_pool(name="pouts", bufs=3))
        ptmp = ctx.enter_context(tc.tile_pool(name="ptmp", bufs=4))
        pres = ctx.enter_context(tc.tile_pool(name="pres", bufs=2))
        pwsb = ctx.enter_context(tc.tile_pool(name="pwsb", bufs=3))
        ppw = ctx.enter_context(tc.tile_pool(name="ppw", bufs=2, space="PSUM"))
        ppo = ctx.enter_context(tc.tile_pool(name="ppo", bufs=2, space="PSUM"))

        cwk = consts.tile([P, KW], bf16)
        nc.scalar.dma_start(out=cwk, in_=cwk_t[:])
        cwo = consts.tile([P, 2 * OUT_F], bf16)
        nc.scalar.dma_start(out=cwo, in_=cwo_t[:])
        cwob = consts.tile([P, OUT_F], bf16)
        nc.scalar.dma_start(out=cwob, in_=cwob_t[:])
        idn = consts.tile([P, P], bf16)
        nc.scalar.dma_start(out=idn, in_=ident_t[:])

        pending = []  # (po, tmp, cts01, outs_g, j, g, ...)

        def emit_out_dma(outs_g, g, j0, j1):
            nc.sync.dma_start(
                out=out_t[
                    :,
                    (g * PAIRS_PER_GROUP + j0) * 2 * OUT_F : (g * PAIRS_PER_GROUP + j1)
                    * 2
                    * OUT_F,
                ].rearrange("p (j t o) -> p j t o", j=j1 - j0, t=2),
                in_=outs_g[:, 0 : j1 - j0],
            )

        def emit_reduce(item):
            po, tmp, cts01, outs_g, j, g, j0, j1, nj, dve_red = item
            po4 = po[:, :, 0:64].rearrange("p t (o il) -> p t o il", il=2)
            if dve_red:
                # balance engines: reduce this pair on the DVE instead, via
                # a 2x-mode fold tree over ih (tmp is [p, t, ih, o, il]),
                # then an il reduce_sum and the compact-cwo po add.
                nc.vector.tensor_add(tmp[:, :, 0:4], tmp[:, :, 0:4], tmp[:, :, 4:8])
                nc.vector.tensor_add(tmp[:, :, 0:2], tmp[:, :, 0:2], tmp[:, :, 2:4])
                nc.vector.tensor_add(tmp[:, :, 0:1], tmp[:, :, 0:1], tmp[:, :, 1:2])
                pred = pres.tile([P, 2, OUT_F], f32)
                nc.vector.reduce_sum(pred[:], tmp[:, :, 0], axis=mybir.AxisListType.X)
                nc.vector.tensor_add(outs_g[:, j], pred[:], po[:, :, 0:OUT_F])
                if j == nj - 1:
                    emit_out_dma(outs_g, g, j0, j1)
                return
            # One reduce matmul per tile t: rhs rows are (ih, o, il),
            # contiguous in tmp; the 64 (o, il)-interleaved accumulators in
            # tile t's bank are each revisited every 64 rows (the il split
            # keeps the accumulate RMW-hazard stall modest); the DVE then
            # folds il=0 + il=1 straight into outs_g (no ACT copy).
            for t in (0, 1):
                ov = po[:, t, 0:64].unsqueeze(1).broadcast_to([P, 8, 64])
                nc.tensor.matmul(
                    ov,
                    idn[:],
                    tmp[:, t].rearrange("p ih o il -> p (ih o il)"),
                    start=False,
                    stop=(t == 1),
                    skip_group_check=True,
                )
            nc.vector.reduce_sum(outs_g[:, j], po4, axis=mybir.AxisListType.X)
            if j == nj - 1:
                emit_out_dma(outs_g, g, j0, j1)

        segments = []
        for g in range(GROUPS):
            if g == 0:
                segments += [(g, 0, 2), (g, 2, 4), (g, 4, 8), (g, 8, PAIRS_PER_GROUP)]
            elif g == GROUPS - 1:
                segments += [(g, 0, 8), (g, 8, 12), (g, 12, 14), (g, 14, PAIRS_PER_GROUP)]
            else:
                segments.append((g, 0, PAIRS_PER_GROUP))
        for g, j0, j1 in segments:
            nj = j1 - j0
            c0 = g * GCOLS + j0 * P
            c1 = g * GCOLS + j1 * P
            ct_g = pct.tile([P, GCOLS], bf16, tag="ct_g")
            nc.sync.dma_start(out=ct_g[0:C1, 0 : nj * P], in_=ct_t[0:C1, c0:c1])
            nc.sync.dma_start(
                out=ct_g[64 : 64 + C1, 0 : nj * P],
                in_=ct_t[C1 : 2 * C1, c0:c1],
            )
            x_g = px.tile([P, PAIRS_PER_GROUP, 2, IN_F], bf16, tag="x_g")
            # very first segment: keep x off the scalar queue (behind consts)
            x_eng = nc.sync if (g == 0 and j0 == 0) else nc.scalar
            x_eng.dma_start(
                out=x_g[:, 0:nj],
                in_=x_t[
                    :, (g * PAIRS_PER_GROUP + j0) * 2 * IN_F : (g * PAIRS_PER_GROUP + j1) * 2 * IN_F
                ].rearrange("p (j t i) -> p j t i", j=nj, t=2),
            )
            outs_g = pouts.tile([P, PAIRS_PER_GROUP, 2, OUT_F], f32, tag="outs_g")

            for j in range(nj):
                gp = g * PAIRS_PER_GROUP + j0 + j
                dve_red = DVE_RED_MOD > 0 and gp % DVE_RED_MOD == DVE_RED_MOD // 2
                act_copy = ACT_COPY
                wpair = ppw.tile([P, 2, KW], f32)
                po = ppo.tile([P, 2, 512], f32)
                cts01 = []
                for t, g0 in enumerate((0, 64)):
                    cts = ct_g[g0 : g0 + C1, j * P : (j + 1) * P]
                    cts01.append(cts)
                    nc.tensor.matmul(
                        wpair[:, t, :],
                        cts,
                        cwk[g0 : g0 + C1, :],
                        start=True,
                        stop=True,
                        tile_position=(g0, 0),
                    )
                    # opens tile t's po accumulation bank for the PE
                    # reduce (64 wide: cwo in even cols, zeros in odd cols
                    # init the il=1 accumulator slots); the DVE path takes
                    # the compact cwob instead.
                    nc.tensor.matmul(
                        po[:, t, 0:OUT_F] if dve_red else po[:, t, 0:64],
                        cts,
                        cwob[g0 : g0 + C1, :] if dve_red else cwo[g0 : g0 + C1, :],
                        start=True,
                        stop=dve_red,
                        skip_group_check=True,
                        tile_position=(g0, 0),
                    )
                # tmp is [p, t, ih, o, il] (i = ih*2 + il); the wpair
                # columns were laid out by the host as colw = ih*64+o*2+il,
                # so the mul's w view collapses to a contiguous stream and
                # the x view to 3 dims with a PACKED last dim (il).  With w
                # copied to SBUF bf16 by the ACT engine, every mul operand
                # is 2-byte packed -> the DVE runs in 2x mode.
                tmp = ptmp.tile([P, 2, 8, OUT_F, 2], bf16)
                if act_copy:
                    wsb = pwsb.tile([P, 2 * KW], bf16)
                    nc.scalar.copy(
                        out=wsb, in_=wpair[:].rearrange("p t k -> p (t k)")
                    )
                    wview = wsb[:].rearrange(
                        "p (t ih o il) -> p t ih o il", t=2, ih=8, il=2
                    )
                else:
                    wview = wpair[:].rearrange(
                        "p t (ih o il) -> p t ih o il", ih=8, il=2
                    )
                xv = (
                    x_g[:, j]
                    .rearrange("p t (ih il) -> p t ih il", il=2)
                    .unsqueeze(3)
                    .broadcast_to([P, 2, 8, OUT_F, 2])
                )
                nc.vector.tensor_mul(tmp[:], wview, xv)
                pending.append((po, tmp, cts01, outs_g, j, g, j0, j1, nj, dve_red))
                if len(pending) > 1:
                    emit_reduce(pending.pop(0))
        while pending:
            emit_reduce(pending.pop(0))

    nc.compile()
    return nc


def _get_nc():
    global _cached_nc
    if _cached_nc is None:
        _cached_nc = _build_nc()
    return _cached_nc


def _make_in_maps(input, cond, cond_weight, cond_bias):
    import ml_dtypes

    bf = ml_dtypes.bfloat16
    ident = np.eye(P, dtype=bf)
    in_maps = []
    n_heads, b_total = input.shape[0], input.shape[1]
    for h in range(n_heads):
        c1t = np.empty((C1, b_total), np.float32)
        c1t[:COND_IN] = cond[h].T
        c1t[COND_IN] = 1.0
        # [33, g, t, s] -> [t, 33, g, s] -> [66, g*s]
        ct = (
            c1t.reshape(C1, GROUPS, 2, GCOLS)
            .transpose(2, 0, 1, 3)
            .reshape(2 * C1, GROUPS * GCOLS)
        )
        ct = np.ascontiguousarray(ct)
        # x[p, (g j t i)] = input[g*4096 + t*2048 + j*128 + p, i]
        x = (
            input[h]
            .reshape(GROUPS, 2, PAIRS_PER_GROUP, P, IN_F)
            .transpose(3, 0, 2, 1, 4)
            .reshape(P, b_total // P * IN_F)
        )
        x = np.ascontiguousarray(x).astype(bf)
        cw3 = cond_weight[h].reshape(OUT_F, INP1, COND_IN)  # (o, i, c)
        cb2 = cond_bias[h].reshape(OUT_F, INP1)  # (o, i)
        cwk = np.zeros((P, KW), np.float32)
        # col = ih*64 + o*2 + il  (i = ih*2 + il)
        cwk1 = (
            cw3[:, :IN_F, :]
            .reshape(OUT_F, 8, 2, COND_IN)
            .transpose(3, 1, 0, 2)
            .reshape(COND_IN, KW)
        )
        cbk = (
            cb2[:, :IN_F].reshape(OUT_F, 8, 2).transpose(1, 0, 2).reshape(KW)
        )
        cwk[0:COND_IN] = cwk1
        cwk[COND_IN] = cbk
        cwk[64 : 64 + COND_IN] = cwk1
        cwk[64 + COND_IN] = cbk
        cwo = np.zeros((P, 2 * OUT_F), np.float32)
        cwo[0:COND_IN, 0 : 2 * OUT_F : 2] = cw3[:, IN_F, :].T  # [c, o]
        cwo[COND_IN, 0 : 2 * OUT_F : 2] = cb2[:, IN_F]
        cwo[64 : 64 + COND_IN, 0 : 2 * OUT_F : 2] = cw3[:, IN_F, :].T
        cwo[64 + COND_IN, 0 : 2 * OUT_F : 2] = cb2[:, IN_F]
        cwob = np.ascontiguousarray(cwo[:, 0 : 2 * OUT_F : 2])
        in_maps.append(
            {
                "ct": ct.astype(bf),
                "x": x,
                "cwk": cwk.astype(bf),
                "cwo": cwo.astype(bf),
                "cwob": cwob.astype(bf),
                "ident": ident,
            }
        )
    return in_maps


def _unpack_out(res):
    # out[p, (g j t o)] -> [g, t, j, p, o] -> [B, o]
    outs = []
    for r in res.results:
        o = (
            r["out"]
            .reshape(P, GROUPS, PAIRS_PER_GROUP, 2, OUT_F)
            .transpose(1, 3, 2, 0, 4)
            .reshape(B, OUT_F)
        )
        outs.append(o)
    return np.stack(outs, axis=0)


def _run(in_maps, **kwargs):
    from concourse import bass_utils

    nc = _get_nc()
    return bass_utils.run_bass_kernel_spmd(
        nc, in_maps, core_ids=list(range(N_HEADS)), **kwargs
    )


def kernel(input, cond, cond_weight, cond_bias):
    input = np.asarray(input, np.float32)
    cond = np.asarray(cond, np.float32)
    cond_weight = np.asarray(cond_weight, np.float32)
    cond_bias = np.asarray(cond_bias, np.float32)
    in_maps = _make_in_maps(input, cond, cond_weight, cond_bias)
    res = _run(in_maps)
    return _unpack_out(res)

